# revision 24
# baseline (speedup 1.0000x reference)
"""Trainium2 Bass kernel for nn_CMAModel (control-fused memory attention).

Math (reference):
  q  = x @ Wq.T + ctrl @ Wc.T                  [B,T,C]
  kv = [x; fwd_mem; rev_mem]                   [B,S,C], S = T+M+R = 5440
  k  = kv @ Wk.T ; v = kv @ Wv.T
  per head h (D=128): scores = q_h k_h^T / sqrt(D), causal mask on the
  local T block only; w = softmax(scores); out_h = w_loc v_loc + gate_h *
  (w_mem v_mem); gate = sigmoid(q @ Wg.T + bg); y = concat(out_h) @ Wo.T

Sharding (8 cores, SPMD — one program, per-core behavior via input data):
  core = b*4 + g  (b = batch, g = group 0..3).  24 units of (b, head,
  T-half).  Each core runs 3 "slots": slots 0,1 = both halves of a
  "pair" head, slot 2 = one half of a "single" head (shared with the
  neighbor core).  Per batch:
    g=0: pair h0, single (h1, half A)     g=1: pair h2, single (h1, B)
    g=2: pair h3, single (h4, half A)     g=3: pair h5, single (h4, B)
  K/V are computed on-device per head-cache (cache0 = pair head,
  cache1 = single head) from the core's batch kv, column-sliced weights.

Layouts: everything feature-major ([C, tokens]) so all matmuls are
  natural (lhsT = transposed weights supplied by the host; no on-device
  transposes).  Attention uses scoresT [s, t]: softmax denominators are
  per-t sums over the s (partition) axis, computed by accumulating
  exp-tiles into a running R on DVE and one ones-vector matmul at the
  end.  Causal masking is (iota >= thr) with host-supplied per-partition
  thresholds — fully data-driven, identical control flow on all cores.

Output: per-slot out-projection partials y_p = Wo[:, h-slice].T-free
  contribution [768, 1024]; the host sums the 6 head partials per
  (batch, half) and transposes — the standard row-parallel unshard.
"""

import numpy as np

B, T, C, H, M, R = 2, 2048, 768, 6, 3072, 320
D = C // H          # 128
S = T + M + R       # 5440
P = 128
NT = (S + P - 1) // P          # 43 s-tiles (last has 64 rows)
NLOC = T // P                  # 16 local s-tiles
NCT = C // P                   # 6 feature tiles
THALF = T // 2                 # 1024
NCH = THALF // 512             # 2 chunks of 512 per half
DSCALE = float(D) ** -0.5

# per-batch slot maps: (pair_head, single_head, single_half) per group
GROUP_MAP = [(0, 1, 0), (2, 1, 1), (3, 4, 0), (5, 4, 1)]


def slot_units(g):
    hp, hs, hsh = GROUP_MAP[g]
    return [(hp, 0), (hp, 1), (hs, hsh)]


def _kchunks():
    out = []
    off = 0
    while off < S:
        w = min(512, S - off)
        out.append((off, w))
        off += w
    return out


KCH = _kchunks()               # 10x512 + 320


def build_nc(use_f32r=True, debug=False, att_bf16=True,
             use_gp_bcast=False):
    import concourse.mybir as mybir
    import concourse.tile as tile
    from concourse import bacc

    f32 = mybir.dt.float32
    f32r = mybir.dt.float32r if use_f32r else f32
    adt = mybir.dt.bfloat16 if att_bf16 else f32r
    AF = mybir.ActivationFunctionType
    OP = mybir.AluOpType

    mdt = f32r

    def mm(psum, lhsT, rhs, start=True, stop=True, rdt=None):
        nc.tensor.matmul(psum, lhsT, rhs, start=start, stop=stop)

    nc = bacc.Bacc("TRN2", target_bir_lowering=False, debug=False,
                   num_devices=8)

    dram = {}
    for name, shape in [
        ("kvT", [C, S]),            # batch kv, transposed
        ("xqT", [C, 3 * THALF]),    # per-slot x columns, transposed
        ("wqT", [C, 3 * P]),        # per-slot Wq head-rows, transposed
        ("wcT_s", [5, 3 * P]),      # per-slot Wc head-rows, transposed
        ("wcT", [5, C]),            # full Wc transposed
        ("wkT0", [C, P]),           # pair-head Wk rows, transposed
        ("wkT1", [C, P]),           # single-head Wk rows, transposed
        ("wvT2", [C, 2 * P]),       # [pair | single] Wv rows, transposed
        ("woT", [P, 3 * C]),        # per-slot Wo head-cols, transposed
        ("wq", [C, C]),             # Wq as-is
        ("wgT", [C, 3]),            # per-slot Wg row, transposed
        ("bg3", [1, 3]),            # per-slot gate bias
        ("ctrl5", [5, 1]),
        ("iota", [P, THALF]),       # iota[i, c] = c
        ("ones_r", [1, P]),         # ones row (f32r bcast stationary)
        ("thr", [P, 3 * NLOC]),    # causal thresholds
    ]:
        dt_ = f32r if name in ("kvT", "xqT", "wqT", "wkT0", "wkT1",
                               "wvT2", "ones_r") else f32
        if name == "woT":
            dt_ = f32 if att_bf16 else f32r
        dram[name] = nc.dram_tensor(name, shape, dt_, kind="ExternalInput")
    yp = nc.dram_tensor("yp", [3 * C, THALF], f32, kind="ExternalOutput")
    dbg = {}
    if debug:
        for name, shape in [("d_q", [P, 3 * THALF]), ("d_gate", [1, 3 * THALF]),
                            ("d_kh0", [P, 1024]), ("d_vh", [P, 512]),
                            ("d_rr", [1, 3 * THALF]),
                            ("d_att", [P, 3 * THALF])]:
            dbg[name] = nc.dram_tensor(name, shape, f32,
                                       kind="ExternalOutput")

    from contextlib import ExitStack

    with tile.TileContext(nc) as tc, ExitStack() as _ctx:
        consts = _ctx.enter_context(tc.tile_pool(name="consts", bufs=1))
        # ---- constants into SBUF ----
        wk0 = consts.tile([P, NCT, P], f32r)
        wk1 = consts.tile([P, NCT, P], f32r)
        wv2 = consts.tile([P, NCT, 2 * P], f32r)
        for ct in range(NCT):
            sl = slice(ct * P, (ct + 1) * P)
            nc.gpsimd.dma_start(out=wk0[:, ct, :], in_=dram["wkT0"][sl, :])
            nc.gpsimd.dma_start(out=wk1[:, ct, :], in_=dram["wkT1"][sl, :])
            nc.gpsimd.dma_start(out=wv2[:, ct, :], in_=dram["wvT2"][sl, :])
        ones_col = consts.tile([P, 1], adt)
        nc.vector.memset(ones_col[:], 1.0)
        ones_row = consts.tile([1, P], f32r)
        nc.sync.dma_start(out=ones_row[:], in_=dram["ones_r"][:, :])

        # ---- phase 2: K/V projections into SBUF caches ----
        kh0 = consts.tile([P, S], adt)
        kh1 = consts.tile([P, S], adt)
        vh = consts.tile([P, NT, 2 * P], adt)
        with tc.tile_pool(name="kvp", bufs=4) as kvp, \
             tc.tile_pool(name="kvps", bufs=1, space="PSUM") as kvps:
            for sc, (off, w) in enumerate(KCH):
                pk0 = kvps.tile([P, 512], f32, tag="k0", bufs=2)
                pk1 = kvps.tile([P, 512], f32, tag="k1", bufs=2)
                subs = []
                o2 = off
                while o2 < off + w:
                    subs.append((o2 - off, min(P, off + w - o2)))
                    o2 += P
                pv = [kvps.tile([P, 2 * P], f32, tag=f"v{si}",
                                name=f"pv{si}", bufs=1)
                      for si in range(len(subs))]
                for ct in range(NCT):
                    kv_t = kvp.tile([P, 512], f32r, tag="kv")
                    nc.sync.dma_start(
                        out=kv_t[:, :w],
                        in_=dram["kvT"][ct * P:(ct + 1) * P, off:off + w])
                    mm(pk0[:, :w], wk0[:, ct, :], kv_t[:, :w],
                       start=(ct == 0), stop=(ct == NCT - 1))
                    mm(pk1[:, :w], wk1[:, ct, :], kv_t[:, :w],
                       start=(ct == 0), stop=(ct == NCT - 1))
                    for si, (so, sw) in enumerate(subs):
                        mm(pv[si][:sw, :], kv_t[:, so:so + sw],
                           wv2[:, ct, :],
                           start=(ct == 0), stop=(ct == NCT - 1))
                nc.vector.tensor_copy(out=kh0[:, off:off + w],
                                      in_=pk0[:, :w])
                nc.vector.tensor_copy(out=kh1[:, off:off + w],
                                      in_=pk1[:, :w])
                for si, (so, sw) in enumerate(subs):
                    j = (off + so) // P
                    nc.vector.tensor_copy(out=vh[:sw, j, :],
                                          in_=pv[si][:sw, :])

        # ---- remaining constants (after the kv stream is queued) ----
        wqt = consts.tile([P, NCT, 3 * P], f32r)
        wgt = consts.tile([P, NCT, 3], f32)
        for ct in range(NCT):
            sl = slice(ct * P, (ct + 1) * P)
            nc.gpsimd.dma_start(out=wqt[:, ct, :], in_=dram["wqT"][sl, :])
            nc.gpsimd.dma_start(out=wgt[:, ct, :], in_=dram["wgT"][sl, :])
        wot = consts.tile([P, 3 * C], adt)
        if att_bf16:
            nc.gpsimd.dma_start(out=wot[:], in_=dram["woT"][:, :])
        else:
            nc.sync.dma_start(out=wot[:], in_=dram["woT"][:, :])
        wct_s = consts.tile([5, 3 * P], f32)
        nc.gpsimd.dma_start(out=wct_s[:], in_=dram["wcT_s"][:, :])
        wct = consts.tile([5, C], f32)
        nc.gpsimd.dma_start(out=wct[:], in_=dram["wcT"][:, :])
        bg3 = consts.tile([1, 3], f32)
        nc.gpsimd.dma_start(out=bg3[:], in_=dram["bg3"][:, :])
        ctrl5 = consts.tile([5, 1], f32)
        nc.gpsimd.dma_start(out=ctrl5[:], in_=dram["ctrl5"][:, :])
        iota = consts.tile([P, THALF], f32)
        nc.gpsimd.dma_start(out=iota[:], in_=dram["iota"][:, :])
        thr = consts.tile([P, 3 * NLOC], f32)
        nc.gpsimd.dma_start(out=thr[:], in_=dram["thr"][:, :])
        # ---- phase 1: tiny precomputes (plain fp32) ----
        qbs = consts.tile([P, 3], f32)      # per-slot q bias column
        qbf = consts.tile([P, NCT], f32)    # full q bias (per c-tile col)
        wfT = consts.tile([P, NCT, 3], f32r)  # fused gate weight cols
        gb3 = consts.tile([1, 3], f32)      # gate bias per slot
        with tc.tile_pool(name="p1w", bufs=1) as p1w, \
             tc.tile_pool(name="p1ps", bufs=2, space="PSUM") as p1ps:
            wqsb = p1w.tile([P, NCT, C], f32)
            for ct in range(NCT):
                nc.gpsimd.dma_start(out=wqsb[:, ct, :],
                                  in_=dram["wq"][ct * P:(ct + 1) * P, :])
            for k in range(3):
                ps = p1ps.tile([P, 1], f32, tag="qb")
                mm(ps[:], wct_s[:, k * P:(k + 1) * P], ctrl5[:], rdt=f32)
                nc.scalar.copy(qbs[:, k:k + 1], ps[:])
            for ct in range(NCT):
                ps = p1ps.tile([P, 1], f32, tag="qb")
                mm(ps[:], wct[:, ct * P:(ct + 1) * P], ctrl5[:], rdt=f32)
                nc.scalar.copy(qbf[:, ct:ct + 1], ps[:])
            for ctp in range(NCT):
                ps = p1ps.tile([P, 3], f32, tag="wf")
                for ct in range(NCT):
                    mm(ps[:], wqsb[:, ct, ctp * P:(ctp + 1) * P],
                       wgt[:, ct, :], start=(ct == 0), stop=(ct == NCT - 1),
                       rdt=f32)
                nc.scalar.copy(wfT[:, ctp, :], ps[:])
            ps = p1ps.tile([1, 3], f32, tag="gb")
            for ct in range(NCT):
                mm(ps[:], qbf[:, ct:ct + 1], wgt[:, ct, :],
                   start=(ct == 0), stop=(ct == NCT - 1), rdt=f32)
            nc.vector.tensor_tensor(gb3[:], ps[:], bg3[:], OP.add)

        # ---- phase 3: q projection + gate ----
        qsb = consts.tile([P, 3, THALF], adt)
        gate = consts.tile([1, 3, THALF], f32)
        with tc.tile_pool(name="xqp", bufs=4) as xqp, \
             tc.tile_pool(name="qps", bufs=1, space="PSUM") as qps:
            for k in range(3):
                for ch in range(NCH):
                    pq = qps.tile([P, 512], f32, tag="q", bufs=2)
                    pg = qps.tile([1, 512], f32, tag="g", bufs=2)
                    for ct in range(NCT):
                        xq_t = xqp.tile([P, 512], f32r, tag="xq")
                        nc.gpsimd.dma_start(
                            out=xq_t[:],
                            in_=dram["xqT"][ct * P:(ct + 1) * P,
                                            k * THALF + ch * 512:
                                            k * THALF + (ch + 1) * 512])
                        mm(pq[:], wqt[:, ct, k * P:(k + 1) * P], xq_t[:],
                           start=(ct == 0), stop=(ct == NCT - 1))
                        mm(pg[:], wfT[:, ct, k:k + 1], xq_t[:],
                           start=(ct == 0), stop=(ct == NCT - 1))
                    nc.vector.tensor_scalar_add(
                        qsb[:, k, ch * 512:(ch + 1) * 512], pq[:],
                        qbs[:, k:k + 1])
                    nc.scalar.activation(
                        gate[0:1, k, ch * 512:(ch + 1) * 512], pg[:],
                        AF.Sigmoid, bias=gb3[0:1, k:k + 1], scale=1.0)

        if debug:
            nc.gpsimd.dma_start(out=dbg["d_q"][:, :],
                               in_=qsb[:].rearrange("p a b -> p (a b)"))
            nc.sync.dma_start(out=dbg["d_gate"][0:1, :],
                              in_=gate[:].rearrange("p a b -> p (a b)"))
            nc.gpsimd.dma_start(out=dbg["d_kh0"][:, :], in_=kh0[:, 0:1024])
            nc.gpsimd.dma_start(out=dbg["d_vh"][:, :],
                               in_=vh[:, 0:2, :].rearrange("p a b -> p (a b)"))
        # ---- phase 4: attention + output projection, per slot ----
        # Per slot: one N=1024 scoresT matmul per s-tile into a bf16
        # PSUM bank, one exp, data-driven causal mask, then per-512-chunk
        # AV accumulation (local/memory f32 psums) and an M=1 ones-matmul
        # denominator.  Slots 0/1 have compile-time halves (dead local
        # s-tiles skipped); slot 2's half is data-dependent, so it runs
        # all local tiles with masks.
        with tc.tile_pool(name="att", bufs=1) as att_pool, \
             tc.tile_pool(name="ep", bufs=6) as ep, \
             tc.tile_pool(name="mp", bufs=3) as mpp, \
             tc.tile_pool(name="vec", bufs=2) as vec, \
             tc.tile_pool(name="cmb", bufs=1) as cmb, \
             tc.tile_pool(name="ysb", bufs=2) as ysb, \
             tc.tile_pool(name="aps", bufs=1, space="PSUM") as aps:
            for k in range(3):
                kh = kh0 if k < 2 else kh1
                voff = 0 if k < 2 else P
                loc_end = 8 if k == 0 else NLOC
                msk_lo = {0: 0, 1: 8, 2: 0}[k]
                js = list(range(loc_end)) + list(range(NLOC, NT))
                attb = att_pool.tile([P, NCH, 512], adt, tag="attb")
                pden = aps.tile([64, 512], f32, tag="den", bufs=2)
                qrhs = qsb[:, k, :]
                pacc = {}    # (ch, 'L'|'M') -> live psum accumulator
                Lsb = att_pool.tile([P, NCH, 512], f32, tag="Lsb")
                Et = {}
                pend = []

                def emit_av(j):
                    spn = min(P, S - j * P)
                    E2 = Et.pop(j)
                    reg = 'L' if j < NLOC else 'M'
                    first = j == 0 or j == NLOC
                    last = j == loc_end - 1 or j == NT - 1
                    for ch in range(NCH):
                        if first:
                            if reg == 'M' and (ch, 'L') in pacc:
                                # retire L into SBUF; M reuses the bank
                                nc.vector.tensor_copy(
                                    out=Lsb[:, ch, :],
                                    in_=pacc.pop((ch, 'L'))[:])
                            pacc[(ch, reg)] = aps.tile(
                                [P, 512], f32, tag=f"a{ch}", name=f"pa{ch}")
                        mm(pacc[(ch, reg)][:], vh[:spn, j, voff:voff + P],
                           E2[:spn, ch * 512:(ch + 1) * 512],
                           start=first, stop=last)
                    for ch in range(NCH):
                        mm(pden[32 * ch:32 * ch + 1, :], ones_col[:spn],
                           E2[:spn, ch * 512:(ch + 1) * 512],
                           start=(j == 0), stop=(j == js[-1]))

                for j in js:
                    spn = min(P, S - j * P)
                    ps = aps.tile([P, NCH, 512], f32, tag="sc", bufs=2)
                    for ch in range(NCH):
                        mm(ps[:spn, ch, :], kh[:, j * P:j * P + spn],
                           qrhs[:, ch * 512:(ch + 1) * 512])
                    E2 = ep.tile([P, THALF], adt, tag="E")
                    nc.scalar.activation(E2[:spn], ps[:spn].rearrange(
                        "p a b -> p (a b)"), AF.Exp, scale=DSCALE)
                    if msk_lo <= j < loc_end:
                        col = k * NLOC + j
                        msk = mpp.tile([P, THALF], adt, tag="msk")
                        nc.vector.tensor_scalar(
                            msk[:spn], iota[:spn],
                            thr[:spn, col:col + 1], None, OP.is_ge)
                        nc.vector.tensor_tensor(E2[:spn], E2[:spn],
                                                msk[:spn], OP.mult)
                    Et[j] = E2
                    pend.append(j)
                    if len(pend) > 3:
                        emit_av(pend.pop(0))
                for j in pend:
                    emit_av(j)

                for ch in range(NCH):
                    rr = vec.tile([1, 512], f32r, tag="rr")
                    with nc.allow_low_precision(reason="f32r normalizers"):
                        nc.vector.reciprocal(rr[:],
                                             pden[32 * ch:32 * ch + 1, :])
                    if debug:
                        nc.gpsimd.dma_start(
                            out=dbg["d_rr"][0:1, k * THALF + ch * 512:
                                            k * THALF + (ch + 1) * 512],
                            in_=rr[:])
                    gr = vec.tile([1, 512], f32r, tag="gr")
                    nc.vector.tensor_tensor(
                        gr[:], gate[0:1, k, ch * 512:(ch + 1) * 512], rr[:],
                        OP.mult)
                    rb = cmb.tile([P, 512], f32, tag="rb")
                    gb = cmb.tile([P, 512], f32, tag="gb")
                    prb = aps.tile([P, 512], f32, tag="sc", bufs=2)
                    mm(prb[:], ones_row[:], rr[:])
                    nc.vector.tensor_copy(out=rb[:], in_=prb[:])
                    pgb = aps.tile([P, 512], f32, tag="sc", bufs=2)
                    mm(pgb[:], ones_row[:], gr[:])
                    nc.vector.tensor_copy(out=gb[:], in_=pgb[:])
                    t1 = cmb.tile([P, 512], f32, tag="t1")
                    nc.vector.tensor_tensor(t1[:], Lsb[:, ch, :], rb[:],
                                            OP.mult)
                    t2 = cmb.tile([P, 512], f32, tag="t2")
                    nc.vector.tensor_tensor(t2[:], pacc.pop((ch, 'M'))[:],
                                            gb[:], OP.mult)
                    nc.vector.tensor_tensor(attb[:, ch, :], t1[:], t2[:],
                                            OP.add)
                if debug:
                    nc.gpsimd.dma_start(
                        out=dbg["d_att"][:, k * THALF:(k + 1) * THALF],
                        in_=attb[:].rearrange("p a b -> p (a b)"))
                for ot in range(NCT):
                    for ch in range(NCH):
                        py = aps.tile([P, 512], f32, tag="sc", bufs=2)
                        mm(py[:], wot[:, k * C + ot * P:k * C + (ot + 1) * P],
                           attb[:, ch, :])
                        yt = ysb.tile([P, 512], f32, tag="y")
                        nc.scalar.copy(yt[:], py[:])
                        nc.sync.dma_start(
                            out=yp[k * C + ot * P:k * C + (ot + 1) * P,
                                   ch * 512:(ch + 1) * 512],
                            in_=yt[:])
    nc.compile()
    return nc


def make_in_maps(x, forward_memory, reverse_memory, ctrl, Wq, Wk, Wv, Wo,
                 Wc, Wg, bg):
    f = np.float32
    iota = np.broadcast_to(np.arange(THALF, dtype=f), (P, THALF)).copy()
    in_maps = []
    for core in range(8):
        b, g = core // 4, core % 4
        units = slot_units(g)
        hp, hs, _ = GROUP_MAP[g]
        kv = np.concatenate(
            [x[b], forward_memory[b], reverse_memory[b]], axis=0)
        kvT = np.ascontiguousarray(kv.T, dtype=f)
        xqT = np.concatenate(
            [np.ascontiguousarray(x[b, h2 * THALF:(h2 + 1) * THALF, :].T)
             for (_, h2) in units], axis=1)
        wqT = np.concatenate(
            [np.ascontiguousarray(Wq[h * P:(h + 1) * P, :].T)
             for (h, _) in units], axis=1)
        wcT_s = np.concatenate(
            [np.ascontiguousarray(Wc[h * P:(h + 1) * P, :].T)
             for (h, _) in units], axis=1)
        wkT0 = np.ascontiguousarray(Wk[hp * P:(hp + 1) * P, :].T)
        wkT1 = np.ascontiguousarray(Wk[hs * P:(hs + 1) * P, :].T)
        wvT2 = np.concatenate(
            [np.ascontiguousarray(Wv[h * P:(h + 1) * P, :].T)
             for h in (hp, hs)], axis=1)
        woT = np.concatenate(
            [np.ascontiguousarray(Wo[:, h * P:(h + 1) * P].T)
             for (h, _) in units], axis=1)
        wgT = np.stack([Wg[h, :] for (h, _) in units], axis=1)
        bg3 = np.array([[bg[h] for (h, _) in units]], dtype=f)
        thr = np.empty((P, 3 * NLOC), dtype=f)
        i = np.arange(P, dtype=f)
        for kslot, (_, half) in enumerate(units):
            for j in range(NLOC):
                thr[:, kslot * NLOC + j] = i + 128 * j - THALF * half
        in_maps.append({
            "kvT": kvT, "xqT": np.ascontiguousarray(xqT, dtype=f),
            "wqT": np.ascontiguousarray(wqT, dtype=f),
            "wcT_s": np.ascontiguousarray(wcT_s, dtype=f),
            "wcT": np.ascontiguousarray(Wc.T, dtype=f),
            "wkT0": wkT0, "wkT1": wkT1,
            "wvT2": np.ascontiguousarray(wvT2, dtype=f),
            "woT": np.ascontiguousarray(woT, dtype=f),
            "wq": np.ascontiguousarray(Wq, dtype=f),
            "wgT": np.ascontiguousarray(wgT, dtype=f),
            "bg3": bg3,
            "ctrl5": np.asarray(ctrl, dtype=f).reshape(5, 1),
            "iota": iota, "thr": thr,
            "ones_r": np.ones((1, P), dtype=f),
        })
    return in_maps


def unshard(results):
    y = np.zeros((B, T, C), dtype=np.float32)
    for core in range(8):
        b, g = core // 4, core % 4
        ypc = results[core]["yp"]
        for kslot, (_, half) in enumerate(slot_units(g)):
            y[b, half * THALF:(half + 1) * THALF, :] += \
                ypc[kslot * C:(kslot + 1) * C, :].T
    return y


_nc_cache = {}


def _get_nc(use_f32r=True, debug=False, att_bf16=True):
    key = (use_f32r, debug, att_bf16)
    if key not in _nc_cache:
        _nc_cache[key] = build_nc(use_f32r, debug, att_bf16)
    return _nc_cache[key]


def kernel(**inputs):
    return kernel_ex(**inputs)[0]


def kernel_ex(trace=False, trace_cores=None, use_f32r=True, debug=False,
              att_bf16=True, **inputs):
    from concourse.bass_utils import run_bass_kernel_spmd

    np_inputs = {k: np.asarray(v) for k, v in inputs.items()}
    in_maps = make_in_maps(**np_inputs)
    nc = _get_nc(use_f32r, debug, att_bf16)
    res = run_bass_kernel_spmd(nc, in_maps, list(range(8)), trace=trace,
                               trace_cores=trace_cores)
    return unshard(res.results), res


# revision 27
# speedup vs baseline: 1.0416x; 1.0416x over previous
"""Trainium2 Bass kernel for nn_CMAModel (control-fused memory attention).

Math (reference):
  q  = x @ Wq.T + ctrl @ Wc.T                  [B,T,C]
  kv = [x; fwd_mem; rev_mem]                   [B,S,C], S = T+M+R = 5440
  k  = kv @ Wk.T ; v = kv @ Wv.T
  per head h (D=128): scores = q_h k_h^T / sqrt(D), causal mask on the
  local T block only; w = softmax(scores); out_h = w_loc v_loc + gate_h *
  (w_mem v_mem); gate = sigmoid(q @ Wg.T + bg); y = concat(out_h) @ Wo.T

Sharding (8 cores, SPMD — one program, per-core behavior via input data):
  core = b*4 + g  (b = batch, g = group 0..3).  24 units of (b, head,
  T-half).  Each core runs 3 "slots": slots 0,1 = both halves of a
  "pair" head, slot 2 = one half of a "single" head (shared with the
  neighbor core).  Per batch:
    g=0: pair h0, single (h1, half A)     g=1: pair h2, single (h1, B)
    g=2: pair h3, single (h4, half A)     g=3: pair h5, single (h4, B)
  K/V are computed on-device per head-cache (cache0 = pair head,
  cache1 = single head) from the core's batch kv, column-sliced weights.

Layouts: everything feature-major ([C, tokens]) so all matmuls are
  natural (lhsT = transposed weights supplied by the host; no on-device
  transposes).  Attention uses scoresT [s, t]: softmax denominators are
  per-t sums over the s (partition) axis, computed by accumulating
  exp-tiles into a running R on DVE and one ones-vector matmul at the
  end.  Causal masking is (iota >= thr) with host-supplied per-partition
  thresholds — fully data-driven, identical control flow on all cores.

Output: per-slot out-projection partials y_p = Wo[:, h-slice].T-free
  contribution [768, 1024]; the host sums the 6 head partials per
  (batch, half) and transposes — the standard row-parallel unshard.
"""

import numpy as np

B, T, C, H, M, R = 2, 2048, 768, 6, 3072, 320
D = C // H          # 128
S = T + M + R       # 5440
P = 128
NT = (S + P - 1) // P          # 43 s-tiles (last has 64 rows)
NLOC = T // P                  # 16 local s-tiles
NCT = C // P                   # 6 feature tiles
THALF = T // 2                 # 1024
NCH = THALF // 512             # 2 chunks of 512 per half
DSCALE = float(D) ** -0.5

# per-batch slot maps: (pair_head, single_head, single_half) per group
GROUP_MAP = [(0, 1, 0), (2, 1, 1), (3, 4, 0), (5, 4, 1)]


def slot_units(g):
    hp, hs, hsh = GROUP_MAP[g]
    return [(hp, 0), (hp, 1), (hs, hsh)]


def _kchunks():
    out = []
    off = 0
    while off < S:
        w = min(512, S - off)
        out.append((off, w))
        off += w
    return out


KCH = _kchunks()               # 10x512 + 320


def build_nc(use_f32r=True, debug=False, att_bf16=True,
             use_gp_bcast=False):
    import concourse.mybir as mybir
    import concourse.tile as tile
    from concourse import bacc

    f32 = mybir.dt.float32
    f32r = mybir.dt.float32r if use_f32r else f32
    adt = mybir.dt.bfloat16 if att_bf16 else f32r
    AF = mybir.ActivationFunctionType
    OP = mybir.AluOpType

    mdt = f32r

    def mm(psum, lhsT, rhs, start=True, stop=True, rdt=None):
        nc.tensor.matmul(psum, lhsT, rhs, start=start, stop=stop)

    nc = bacc.Bacc("TRN2", target_bir_lowering=False, debug=False,
                   num_devices=8)

    dram = {}
    for name, shape in [
        ("kvT", [C, S]),            # batch kv, transposed
        ("xqT", [C, 3 * THALF]),    # per-slot x columns, transposed
        ("wqT", [C, 3 * P]),        # per-slot Wq head-rows, transposed
        ("wcT_s", [5, 3 * P]),      # per-slot Wc head-rows, transposed
        ("wcT", [5, C]),            # full Wc transposed
        ("wkT0", [C, P]),           # pair-head Wk rows, transposed
        ("wkT1", [C, P]),           # single-head Wk rows, transposed
        ("wvT2", [C, 2 * P]),       # [pair | single] Wv rows, transposed
        ("woT", [P, 3 * C]),        # per-slot Wo head-cols, transposed
        ("wq", [C, C]),             # Wq as-is
        ("wgT", [C, 3]),            # per-slot Wg row, transposed
        ("bg3", [1, 3]),            # per-slot gate bias
        ("ctrl5", [5, 1]),
        ("iota", [P, THALF]),       # iota[i, c] = c
        ("ones_r", [1, P]),         # ones row (f32r bcast stationary)
        ("ones_c16", [P, 1]),       # fp16 ones col (R reduction)
        ("thr", [P, 3 * NLOC]),    # causal thresholds
    ]:
        dt_ = f32r if name in ("kvT", "xqT", "wqT", "wkT0", "wkT1",
                               "wvT2", "ones_r") else f32
        if name == "woT":
            dt_ = f32 if att_bf16 else f32r
        if name == "ones_c16":
            dt_ = mybir.dt.float16
        dram[name] = nc.dram_tensor(name, shape, dt_, kind="ExternalInput")
    yp = nc.dram_tensor("yp", [3 * C, THALF], f32, kind="ExternalOutput")
    dbg = {}
    if debug:
        for name, shape in [("d_q", [P, 3 * THALF]), ("d_gate", [1, 3 * THALF]),
                            ("d_kh0", [P, 1024]), ("d_vh", [P, 512]),
                            ("d_rr", [1, 3 * THALF]),
                            ("d_att", [P, 3 * THALF])]:
            dbg[name] = nc.dram_tensor(name, shape, f32,
                                       kind="ExternalOutput")

    from contextlib import ExitStack

    with tile.TileContext(nc) as tc, ExitStack() as _ctx:
        consts = _ctx.enter_context(tc.tile_pool(name="consts", bufs=1))
        # ---- constants into SBUF ----
        wk0 = consts.tile([P, NCT, P], f32r)
        wk1 = consts.tile([P, NCT, P], f32r)
        wv2 = consts.tile([P, NCT, 2 * P], f32r)
        for ct in range(NCT):
            sl = slice(ct * P, (ct + 1) * P)
            nc.gpsimd.dma_start(out=wk0[:, ct, :], in_=dram["wkT0"][sl, :])
            nc.gpsimd.dma_start(out=wk1[:, ct, :], in_=dram["wkT1"][sl, :])
            nc.gpsimd.dma_start(out=wv2[:, ct, :], in_=dram["wvT2"][sl, :])
        ones_col = consts.tile([P, 1], adt)
        nc.vector.memset(ones_col[:], 1.0)
        ones_row = consts.tile([1, P], f32r)
        nc.sync.dma_start(out=ones_row[:], in_=dram["ones_r"][:, :])
        ones_c16 = consts.tile([P, 1], mybir.dt.float16)
        nc.sync.dma_start(out=ones_c16[:], in_=dram["ones_c16"][:, :])

        # ---- phase 2: K/V projections into SBUF caches ----
        kh0 = consts.tile([P, S], adt)
        kh1 = consts.tile([P, S], adt)
        vh = consts.tile([P, NT, 2 * P], adt)
        with tc.tile_pool(name="kvp", bufs=4) as kvp, \
             tc.tile_pool(name="kvps", bufs=1, space="PSUM") as kvps:
            for sc, (off, w) in enumerate(KCH):
                pk0 = kvps.tile([P, 512], f32, tag="k0", bufs=2)
                pk1 = kvps.tile([P, 512], f32, tag="k1", bufs=2)
                subs = []
                o2 = off
                while o2 < off + w:
                    subs.append((o2 - off, min(P, off + w - o2)))
                    o2 += P
                pv = [kvps.tile([P, 2 * P], f32, tag=f"v{si}",
                                name=f"pv{si}", bufs=1)
                      for si in range(len(subs))]
                for ct in range(NCT):
                    kv_t = kvp.tile([P, 512], f32r, tag="kv")
                    nc.sync.dma_start(
                        out=kv_t[:, :w],
                        in_=dram["kvT"][ct * P:(ct + 1) * P, off:off + w])
                    mm(pk0[:, :w], wk0[:, ct, :], kv_t[:, :w],
                       start=(ct == 0), stop=(ct == NCT - 1))
                    mm(pk1[:, :w], wk1[:, ct, :], kv_t[:, :w],
                       start=(ct == 0), stop=(ct == NCT - 1))
                    for si, (so, sw) in enumerate(subs):
                        mm(pv[si][:sw, :], kv_t[:, so:so + sw],
                           wv2[:, ct, :],
                           start=(ct == 0), stop=(ct == NCT - 1))
                nc.vector.tensor_copy(out=kh0[:, off:off + w],
                                      in_=pk0[:, :w])
                nc.vector.tensor_copy(out=kh1[:, off:off + w],
                                      in_=pk1[:, :w])
                for si, (so, sw) in enumerate(subs):
                    j = (off + so) // P
                    nc.vector.tensor_copy(out=vh[:sw, j, :],
                                          in_=pv[si][:sw, :])

        # ---- remaining constants (after the kv stream is queued) ----
        wqt = consts.tile([P, NCT, 3 * P], f32r)
        wgt = consts.tile([P, NCT, 3], f32)
        for ct in range(NCT):
            sl = slice(ct * P, (ct + 1) * P)
            nc.gpsimd.dma_start(out=wqt[:, ct, :], in_=dram["wqT"][sl, :])
            nc.gpsimd.dma_start(out=wgt[:, ct, :], in_=dram["wgT"][sl, :])
        wot = consts.tile([P, 3 * C], adt)
        if att_bf16:
            nc.gpsimd.dma_start(out=wot[:], in_=dram["woT"][:, :])
        else:
            nc.sync.dma_start(out=wot[:], in_=dram["woT"][:, :])
        wct_s = consts.tile([5, 3 * P], f32)
        nc.gpsimd.dma_start(out=wct_s[:], in_=dram["wcT_s"][:, :])
        wct = consts.tile([5, C], f32)
        nc.gpsimd.dma_start(out=wct[:], in_=dram["wcT"][:, :])
        bg3 = consts.tile([1, 3], f32)
        nc.gpsimd.dma_start(out=bg3[:], in_=dram["bg3"][:, :])
        ctrl5 = consts.tile([5, 1], f32)
        nc.gpsimd.dma_start(out=ctrl5[:], in_=dram["ctrl5"][:, :])
        iota = consts.tile([P, THALF], f32)
        nc.gpsimd.dma_start(out=iota[:], in_=dram["iota"][:, :])
        thr = consts.tile([P, 3 * NLOC], f32)
        nc.gpsimd.dma_start(out=thr[:], in_=dram["thr"][:, :])
        # ---- phase 1: tiny precomputes (plain fp32) ----
        qbs = consts.tile([P, 3], f32)      # per-slot q bias column
        qbf = consts.tile([P, NCT], f32)    # full q bias (per c-tile col)
        wfT = consts.tile([P, NCT, 3], f32r)  # fused gate weight cols
        gb3 = consts.tile([1, 3], f32)      # gate bias per slot
        with tc.tile_pool(name="p1w", bufs=1) as p1w, \
             tc.tile_pool(name="p1ps", bufs=2, space="PSUM") as p1ps:
            wqsb = p1w.tile([P, NCT, C], f32)
            for ct in range(NCT):
                nc.gpsimd.dma_start(out=wqsb[:, ct, :],
                                  in_=dram["wq"][ct * P:(ct + 1) * P, :])
            for k in range(3):
                ps = p1ps.tile([P, 1], f32, tag="qb")
                mm(ps[:], wct_s[:, k * P:(k + 1) * P], ctrl5[:], rdt=f32)
                nc.scalar.copy(qbs[:, k:k + 1], ps[:])
            for ct in range(NCT):
                ps = p1ps.tile([P, 1], f32, tag="qb")
                mm(ps[:], wct[:, ct * P:(ct + 1) * P], ctrl5[:], rdt=f32)
                nc.scalar.copy(qbf[:, ct:ct + 1], ps[:])
            for ctp in range(NCT):
                ps = p1ps.tile([P, 3], f32, tag="wf")
                for ct in range(NCT):
                    mm(ps[:], wqsb[:, ct, ctp * P:(ctp + 1) * P],
                       wgt[:, ct, :], start=(ct == 0), stop=(ct == NCT - 1),
                       rdt=f32)
                nc.scalar.copy(wfT[:, ctp, :], ps[:])
            ps = p1ps.tile([1, 3], f32, tag="gb")
            for ct in range(NCT):
                mm(ps[:], qbf[:, ct:ct + 1], wgt[:, ct, :],
                   start=(ct == 0), stop=(ct == NCT - 1), rdt=f32)
            nc.vector.tensor_tensor(gb3[:], ps[:], bg3[:], OP.add)

        # ---- phase 3: q projection + gate ----
        qsb = consts.tile([P, 3, THALF], adt)
        gate = consts.tile([1, 3, THALF], f32)
        with tc.tile_pool(name="xqp", bufs=4) as xqp, \
             tc.tile_pool(name="qps", bufs=1, space="PSUM") as qps:
            for k in range(3):
                for ch in range(NCH):
                    pq = qps.tile([P, 512], f32, tag="q", bufs=2)
                    pg = qps.tile([1, 512], f32, tag="g", bufs=2)
                    for ct in range(NCT):
                        xq_t = xqp.tile([P, 512], f32r, tag="xq")
                        nc.gpsimd.dma_start(
                            out=xq_t[:],
                            in_=dram["xqT"][ct * P:(ct + 1) * P,
                                            k * THALF + ch * 512:
                                            k * THALF + (ch + 1) * 512])
                        mm(pq[:], wqt[:, ct, k * P:(k + 1) * P], xq_t[:],
                           start=(ct == 0), stop=(ct == NCT - 1))
                        mm(pg[:], wfT[:, ct, k:k + 1], xq_t[:],
                           start=(ct == 0), stop=(ct == NCT - 1))
                    nc.vector.tensor_scalar_add(
                        qsb[:, k, ch * 512:(ch + 1) * 512], pq[:],
                        qbs[:, k:k + 1])
                    nc.scalar.activation(
                        gate[0:1, k, ch * 512:(ch + 1) * 512], pg[:],
                        AF.Sigmoid, bias=gb3[0:1, k:k + 1], scale=1.0)

        if debug:
            nc.gpsimd.dma_start(out=dbg["d_q"][:, :],
                               in_=qsb[:].rearrange("p a b -> p (a b)"))
            nc.sync.dma_start(out=dbg["d_gate"][0:1, :],
                              in_=gate[:].rearrange("p a b -> p (a b)"))
            nc.gpsimd.dma_start(out=dbg["d_kh0"][:, :], in_=kh0[:, 0:1024])
            nc.gpsimd.dma_start(out=dbg["d_vh"][:, :],
                               in_=vh[:, 0:2, :].rearrange("p a b -> p (a b)"))
        # ---- phase 4: attention + output projection, per slot ----
        # Per slot: one N=1024 scoresT matmul per s-tile into a bf16
        # PSUM bank, one exp, data-driven causal mask, then per-512-chunk
        # AV accumulation (local/memory f32 psums) and an M=1 ones-matmul
        # denominator.  Slots 0/1 have compile-time halves (dead local
        # s-tiles skipped); slot 2's half is data-dependent, so it runs
        # all local tiles with masks.
        with tc.tile_pool(name="att", bufs=1) as att_pool, \
             tc.tile_pool(name="ep", bufs=6) as ep, \
             tc.tile_pool(name="mp", bufs=3) as mpp, \
             tc.tile_pool(name="vec", bufs=2) as vec, \
             tc.tile_pool(name="cmb", bufs=1) as cmb, \
             tc.tile_pool(name="ysb", bufs=2) as ysb, \
             tc.tile_pool(name="aps", bufs=1, space="PSUM") as aps:
            for k in range(3):
                kh = kh0 if k < 2 else kh1
                voff = 0 if k < 2 else P
                loc_end = 8 if k == 0 else NLOC
                msk_lo = {0: 0, 1: 8, 2: 0}[k]
                js = list(range(loc_end)) + list(range(NLOC, NT))
                attb = att_pool.tile([P, NCH, 512], adt, tag="attb")
                Rt = vec.tile([P, NCH, 512], mybir.dt.float16, tag="R")
                qrhs = qsb[:, k, :]
                pacc = {}    # (ch, 'L'|'M') -> live psum accumulator
                Lsb = att_pool.tile([P, NCH, 512], f32, tag="Lsb")
                Et = {}
                pend = []

                def emit_av(j):
                    spn = min(P, S - j * P)
                    E2 = Et.pop(j)
                    reg = 'L' if j < NLOC else 'M'
                    first = j == 0 or j == NLOC
                    last = j == loc_end - 1 or j == NT - 1
                    for ch in range(NCH):
                        if first:
                            if reg == 'M' and (ch, 'L') in pacc:
                                # retire L into SBUF; M reuses the bank
                                nc.vector.tensor_copy(
                                    out=Lsb[:, ch, :],
                                    in_=pacc.pop((ch, 'L'))[:])
                            pacc[(ch, reg)] = aps.tile(
                                [P, 512], f32, tag=f"a{ch}", name=f"pa{ch}")
                        mm(pacc[(ch, reg)][:], vh[:spn, j, voff:voff + P],
                           E2[:spn, ch * 512:(ch + 1) * 512],
                           start=first, stop=last)

                for j in js:
                    spn = min(P, S - j * P)
                    ps = aps.tile([P, NCH, 512], f32, tag="sc", bufs=2)
                    for ch in range(NCH):
                        mm(ps[:spn, ch, :], kh[:, j * P:j * P + spn],
                           qrhs[:, ch * 512:(ch + 1) * 512])
                    E2 = ep.tile([P, THALF], adt, tag="E")
                    nc.scalar.activation(E2[:spn], ps[:spn].rearrange(
                        "p a b -> p (a b)"), AF.Exp, scale=DSCALE)
                    if msk_lo <= j < loc_end:
                        col = k * NLOC + j
                        msk = mpp.tile([P, THALF], adt, tag="msk")
                        nc.vector.tensor_scalar(
                            msk[:spn], iota[:spn],
                            thr[:spn, col:col + 1], None, OP.is_ge)
                        nc.vector.tensor_tensor(E2[:spn], E2[:spn],
                                                msk[:spn], OP.mult)
                    for ch in range(NCH):
                        if j == 0:
                            nc.vector.tensor_copy(
                                out=Rt[:, ch, :],
                                in_=E2[:, ch * 512:(ch + 1) * 512])
                        else:
                            nc.vector.tensor_tensor(
                                Rt[:spn, ch, :], Rt[:spn, ch, :],
                                E2[:spn, ch * 512:(ch + 1) * 512], OP.add)
                    Et[j] = E2
                    pend.append(j)
                    if len(pend) > 3:
                        emit_av(pend.pop(0))
                for j in pend:
                    emit_av(j)

                for ch in range(NCH):
                    pden = aps.tile([1, 512], f32, tag="den", bufs=2)
                    mm(pden[:], ones_c16[:], Rt[:, ch, :])
                    rr = vec.tile([1, 512], f32r, tag="rr")
                    with nc.allow_low_precision(reason="f32r normalizers"):
                        nc.vector.reciprocal(rr[:], pden[:])
                    if debug:
                        nc.gpsimd.dma_start(
                            out=dbg["d_rr"][0:1, k * THALF + ch * 512:
                                            k * THALF + (ch + 1) * 512],
                            in_=rr[:])
                    gr = vec.tile([1, 512], f32r, tag="gr")
                    nc.vector.tensor_tensor(
                        gr[:], gate[0:1, k, ch * 512:(ch + 1) * 512], rr[:],
                        OP.mult)
                    rb = cmb.tile([P, 512], f32, tag="rb")
                    gb = cmb.tile([P, 512], f32, tag="gb")
                    prb = aps.tile([P, 512], f32, tag="sc", bufs=2)
                    mm(prb[:], ones_row[:], rr[:])
                    nc.vector.tensor_copy(out=rb[:], in_=prb[:])
                    pgb = aps.tile([P, 512], f32, tag="sc", bufs=2)
                    mm(pgb[:], ones_row[:], gr[:])
                    nc.vector.tensor_copy(out=gb[:], in_=pgb[:])
                    t1 = cmb.tile([P, 512], f32, tag="t1")
                    nc.vector.tensor_tensor(t1[:], Lsb[:, ch, :], rb[:],
                                            OP.mult)
                    t2 = cmb.tile([P, 512], f32, tag="t2")
                    nc.vector.tensor_tensor(t2[:], pacc.pop((ch, 'M'))[:],
                                            gb[:], OP.mult)
                    nc.vector.tensor_tensor(attb[:, ch, :], t1[:], t2[:],
                                            OP.add)
                if debug:
                    nc.gpsimd.dma_start(
                        out=dbg["d_att"][:, k * THALF:(k + 1) * THALF],
                        in_=attb[:].rearrange("p a b -> p (a b)"))
                for ot in range(NCT):
                    for ch in range(NCH):
                        py = aps.tile([P, 512], f32, tag="sc", bufs=2)
                        mm(py[:], wot[:, k * C + ot * P:k * C + (ot + 1) * P],
                           attb[:, ch, :])
                        yt = ysb.tile([P, 512], f32, tag="y")
                        nc.scalar.copy(yt[:], py[:])
                        nc.sync.dma_start(
                            out=yp[k * C + ot * P:k * C + (ot + 1) * P,
                                   ch * 512:(ch + 1) * 512],
                            in_=yt[:])
    nc.compile()
    return nc


def make_in_maps(x, forward_memory, reverse_memory, ctrl, Wq, Wk, Wv, Wo,
                 Wc, Wg, bg):
    f = np.float32
    iota = np.broadcast_to(np.arange(THALF, dtype=f), (P, THALF)).copy()
    in_maps = []
    for core in range(8):
        b, g = core // 4, core % 4
        units = slot_units(g)
        hp, hs, _ = GROUP_MAP[g]
        kv = np.concatenate(
            [x[b], forward_memory[b], reverse_memory[b]], axis=0)
        kvT = np.ascontiguousarray(kv.T, dtype=f)
        xqT = np.concatenate(
            [np.ascontiguousarray(x[b, h2 * THALF:(h2 + 1) * THALF, :].T)
             for (_, h2) in units], axis=1)
        wqT = np.concatenate(
            [np.ascontiguousarray(Wq[h * P:(h + 1) * P, :].T)
             for (h, _) in units], axis=1)
        wcT_s = np.concatenate(
            [np.ascontiguousarray(Wc[h * P:(h + 1) * P, :].T)
             for (h, _) in units], axis=1)
        wkT0 = np.ascontiguousarray(Wk[hp * P:(hp + 1) * P, :].T)
        wkT1 = np.ascontiguousarray(Wk[hs * P:(hs + 1) * P, :].T)
        wvT2 = np.concatenate(
            [np.ascontiguousarray(Wv[h * P:(h + 1) * P, :].T)
             for h in (hp, hs)], axis=1)
        woT = np.concatenate(
            [np.ascontiguousarray(Wo[:, h * P:(h + 1) * P].T)
             for (h, _) in units], axis=1)
        wgT = np.stack([Wg[h, :] for (h, _) in units], axis=1)
        bg3 = np.array([[bg[h] for (h, _) in units]], dtype=f)
        thr = np.empty((P, 3 * NLOC), dtype=f)
        i = np.arange(P, dtype=f)
        for kslot, (_, half) in enumerate(units):
            for j in range(NLOC):
                thr[:, kslot * NLOC + j] = i + 128 * j - THALF * half
        in_maps.append({
            "kvT": kvT, "xqT": np.ascontiguousarray(xqT, dtype=f),
            "wqT": np.ascontiguousarray(wqT, dtype=f),
            "wcT_s": np.ascontiguousarray(wcT_s, dtype=f),
            "wcT": np.ascontiguousarray(Wc.T, dtype=f),
            "wkT0": wkT0, "wkT1": wkT1,
            "wvT2": np.ascontiguousarray(wvT2, dtype=f),
            "woT": np.ascontiguousarray(woT, dtype=f),
            "wq": np.ascontiguousarray(Wq, dtype=f),
            "wgT": np.ascontiguousarray(wgT, dtype=f),
            "bg3": bg3,
            "ctrl5": np.asarray(ctrl, dtype=f).reshape(5, 1),
            "iota": iota, "thr": thr,
            "ones_r": np.ones((1, P), dtype=f),
            "ones_c16": np.ones((P, 1), dtype=np.float16),
        })
    return in_maps


def unshard(results):
    y = np.zeros((B, T, C), dtype=np.float32)
    for core in range(8):
        b, g = core // 4, core % 4
        ypc = results[core]["yp"]
        for kslot, (_, half) in enumerate(slot_units(g)):
            y[b, half * THALF:(half + 1) * THALF, :] += \
                ypc[kslot * C:(kslot + 1) * C, :].T
    return y


_nc_cache = {}


def _get_nc(use_f32r=True, debug=False, att_bf16=True):
    key = (use_f32r, debug, att_bf16)
    if key not in _nc_cache:
        _nc_cache[key] = build_nc(use_f32r, debug, att_bf16)
    return _nc_cache[key]


def kernel(**inputs):
    return kernel_ex(**inputs)[0]


def kernel_ex(trace=False, trace_cores=None, use_f32r=True, debug=False,
              att_bf16=True, **inputs):
    from concourse.bass_utils import run_bass_kernel_spmd

    np_inputs = {k: np.asarray(v) for k, v in inputs.items()}
    in_maps = make_in_maps(**np_inputs)
    nc = _get_nc(use_f32r, debug, att_bf16)
    res = run_bass_kernel_spmd(nc, in_maps, list(range(8)), trace=trace,
                               trace_cores=trace_cores)
    return unshard(res.results), res
